# revision 31
# baseline (speedup 1.0000x reference)
"""Distributed Bass/Trainium2 kernel for the DKL MMD loss function.

Problem: nn_DKL_70970039599698. Full inputs X,Y [4096,1024], W [1024,128],
b [128], scalars epsilon/sigma_K/sigma_q. The reference computes (note it
uses featY for BOTH feature sides, so all three feature-distance matrices
are the same matrix Df):

  Kx  = (1-e)*exp(-Df/sK - Dxx/sq) + e*exp(-Dxx/sq)
  Ky  = (1-e)*exp(-Df/sK - Dyy/sq) + e*exp(-Dyy/sq)
  Kxy = (1-e)*exp(-Df/sK - Dxy/sq) + e*exp(-Dxy/sq)

and returns (-mmd2, sqrt(varEst+1e-8), ratio) built from sums/traces/row
sums of those three 4096x4096 matrices.

Sharding: each of the 8 cores owns a 512-row band i of the pair grid and
streams all 4096 columns j (512-column blocks). Everything needed
downstream is only:
  rowP_i = sum_j (Kx+Ky-Kxy)[i,j]     (per-core [128,32] partials)
  rowQ_i = sum_j Kxy[i,j]             (per-core [128,32] partials)
  colQ_j = sum_{i in shard} Kxy[i,j]  (per-core partial [1,4096])
which the host all-reduces / concatenates and combines in float64.

The host hands every tensor pre-transposed ([dim, rows] layout) so the
device never transposes: Gram operands DMA straight into SBUF in matmul
layout. Tiles are [i(128 part) x j(512 free)]: the i-side norm factor
rides the ACT exp bias slot, j-side factors are replicated [128,512]
tiles built by ones-matrix matmuls over gpsimd-squared chunks, row sums
come fused out of the DVE chain (scalar_tensor_tensor accum_out) and
col sums are per-block ones-vector matmuls. Grams run in float32r
(TF32, full PE rate; the PE rounds its inputs internally); norms stay
fp32. relu clamping of pdist2 is skipped on device: min off-diagonal
distances are >=1e-3 so the clamp only ever fires on the (noise-level)
diagonals, which the host handles exactly via trace terms.
"""

import os
from contextlib import ExitStack

import numpy as np

import concourse.bacc as bacc
import concourse.bass as bass
import concourse.mybir as mybir
import concourse.tile as tile
from concourse import bass_utils

N = 4096
D = 1024
DF = 128
NCORES = 8
SHARD = N // NCORES  # 512
NBLK = N // 512  # 8 column blocks of 512
F32 = mybir.dt.float32
F32R = mybir.dt.float32 if os.environ.get("NO_F32R") else mybir.dt.float32r
AF = mybir.ActivationFunctionType
OP = mybir.AluOpType

_CACHE: dict = {}
LAST_RESULTS = None  # BassKernelResults of the most recent run (for test.py)


def _build_program(sq: float, sK: float, eps: float):
    nc = bacc.Bacc(trn_type="TRN2", target_bir_lowering=False, debug=False)

    # all row-data inputs arrive pre-transposed: [dim, rows]
    xst_d = nc.dram_tensor("xst", [D, SHARD], F32R, kind="ExternalInput").ap()
    yst_d = nc.dram_tensor("yst", [D, SHARD], F32R, kind="ExternalInput").ap()
    xft_d = nc.dram_tensor("xft", [D, N], F32R, kind="ExternalInput").ap()
    yft_d = nc.dram_tensor("yft", [D, N], F32R, kind="ExternalInput").ap()
    w_d = nc.dram_tensor("w", [D, DF], F32R, kind="ExternalInput").ap()
    b_d = nc.dram_tensor("bvec", [DF, 1], F32, kind="ExternalInput").ap()
    rowp_d = nc.dram_tensor("rowp", [128, 4 * NBLK], F32, kind="ExternalOutput").ap()
    rowq_d = nc.dram_tensor("rowq", [128, 4 * NBLK], F32, kind="ExternalOutput").ap()
    colq_d = nc.dram_tensor("colq", [1, N], F32, kind="ExternalOutput").ap()

    use_pool = not os.environ.get("NO_POOL_CHAIN")

    with tile.TileContext(nc) as tc, ExitStack() as ctx:
        consts = ctx.enter_context(tc.tile_pool(name="consts", bufs=1))
        persist = ctx.enter_context(tc.tile_pool(name="persist", bufs=1))
        xtp = ctx.enter_context(tc.tile_pool(name="xtp", bufs=2))
        scrp = ctx.enter_context(tc.tile_pool(name="scrp", bufs=13))
        repp = ctx.enter_context(tc.tile_pool(name="repp", bufs=2))
        ftp = ctx.enter_context(tc.tile_pool(name="ftp", bufs=2))
        expp = ctx.enter_context(tc.tile_pool(name="expp", bufs=2))
        workp = ctx.enter_context(tc.tile_pool(name="workp", bufs=2))
        qrp = ctx.enter_context(tc.tile_pool(name="qrp", bufs=4))
        smallp = ctx.enter_context(tc.tile_pool(name="smallp", bufs=2))
        tpsum = ctx.enter_context(tc.tile_pool(name="tpsum", bufs=3, space="PSUM"))
        cqp = ctx.enter_context(tc.tile_pool(name="cqp", bufs=1, space="PSUM"))
        gpsum = ctx.enter_context(tc.tile_pool(name="gpsum", bufs=1, space="PSUM"))

        ones1 = consts.tile([1, 128], F32, tag="ones1")
        nc.gpsimd.memset(ones1[:], 1.0)
        ones128 = consts.tile([128, 1], F32, tag="ones128")
        nc.gpsimd.memset(ones128[:], 1.0)
        ones_r = consts.tile([128, 1], F32R, tag="ones_r")
        nc.vector.tensor_copy(ones_r[:], ones128[:])
        onesmat = consts.tile([128, 128], F32, tag="onesmat")
        nc.gpsimd.memset(onesmat[:], 1.0)
        onesmat_r = consts.tile([128, 128], F32R, tag="onesmat_r")
        nc.vector.tensor_copy(onesmat_r[:], onesmat[:])
        w_r = consts.tile([128, 8, 128], F32R, tag="w_r")
        nc.sync.dma_start(w_r[:], w_d.rearrange("(c p) f -> p c f", p=128))
        b_sb = consts.tile([128, 1], F32, tag="b_sb")
        nc.sync.dma_start(b_sb[:], b_d[:])

        # ---------------- phase 1: shard prep ----------------
        xst = persist.tile([128, 8, SHARD], F32R, tag="xst")  # [dim, kchunk, i]
        yst = persist.tile([128, 8, SHARD], F32R, tag="yst")
        for h in range(4):
            hs = slice(h * 2, (h + 1) * 2)
            nc.sync.dma_start(
                yst[:, hs, :],
                yst_d.rearrange("(c p) i -> p c i", p=128)[:, hs, :],
            )
        for h in range(4):
            hs = slice(h * 2, (h + 1) * 2)
            nc.sync.dma_start(
                xst[:, hs, :],
                xst_d.rearrange("(c p) i -> p c i", p=128)[:, hs, :],
            )

        # fsT: [feat, i] feature block for the shard (features of Y rows)
        fsT = persist.tile([128, SHARD], F32, tag="fsT")
        pf = tpsum.tile([128, SHARD], F32, tag="t")
        for kc in range(8):
            nc.tensor.matmul(
                pf[:], w_r[:, kc, :], yst[:, kc, :], start=(kc == 0), stop=(kc == 7)
            )
        nc.scalar.activation(fsT[:], pf[:], AF.Identity, bias=b_sb[:])
        fsT_r = persist.tile([128, SHARD], F32R, tag="fsT_r")
        nc.vector.tensor_copy(fsT_r[:], fsT[:])

        # shard norms -> [1,512] free -> per-i-chunk [128,1] exp biases
        biases = {}
        for name, src, nchunk, sigma in (
            ("x", xst, 8, sq),
            ("y", yst, 8, sq),
            ("f", fsT_r, 1, sK),
        ):
            ps_n = tpsum.tile([128, SHARD], F32, tag="t")
            for kc in range(nchunk):
                scr = scrp.tile([128, SHARD], F32R, tag="scr")
                chunk = src[:, kc, :] if nchunk > 1 else src[:]
                nc.gpsimd.tensor_tensor(scr[:], chunk, chunk, OP.mult)
                nc.tensor.matmul(
                    ps_n[0:1, :], ones_r[:], scr[:],
                    start=(kc == 0), stop=(kc == nchunk - 1),
                )
            nv = persist.tile([1, SHARD], F32, tag=f"nv_{name}")
            nc.vector.tensor_scalar_mul(nv[:], ps_n[0:1, :], -1.0 / sigma)
            # [1,512] -> 4x [128,1] via K=1 outer-product matmuls
            bias = persist.tile([128, 4], F32, tag=f"bias_{name}")
            for ic in range(4):
                ps_b = tpsum.tile([128, SHARD], F32, tag="t")
                nc.tensor.matmul(
                    ps_b[:, 0:1],
                    nv[0:1, ic * 128 : (ic + 1) * 128],
                    ones1[0:1, 0:1],
                    start=True,
                    stop=True,
                )
                nc.vector.tensor_copy(bias[:, ic : ic + 1], ps_b[:, 0:1])
            biases[name] = bias

        rowp_sb = persist.tile([128, 4 * NBLK], F32, tag="rowp_sb")
        rowq_sb = persist.tile([128, 4 * NBLK], F32, tag="rowq_sb")
        colq_sb = persist.tile([1, N], F32, tag="colq_sb")

        # ---------------- phase 2: stream column blocks ----------------
        for b in range(NBLK):
            j0 = b * 512
            xt = xtp.tile([128, 8, 512], F32R, tag="xt")  # [dim, kchunk, j]
            yt = xtp.tile([128, 8, 512], F32R, tag="yt")
            for h in range(4):
                hs = slice(h * 2, (h + 1) * 2)
                nc.sync.dma_start(
                    yt[:, hs, :],
                    yft_d[:, j0 : j0 + 512].rearrange(
                        "(c p) j -> p c j", p=128
                    )[:, hs, :],
                )
            for h in range(4):
                hs = slice(h * 2, (h + 1) * 2)
                nc.sync.dma_start(
                    xt[:, hs, :],
                    xft_d[:, j0 : j0 + 512].rearrange(
                        "(c p) j -> p c j", p=128
                    )[:, hs, :],
                )

            # F^T for this block ([feat, j])
            pf = tpsum.tile([128, 512], F32, tag="t")
            for kc in range(8):
                nc.tensor.matmul(
                    pf[:], w_r[:, kc, :], yt[:, kc, :], start=(kc == 0), stop=(kc == 7)
                )
            ft = ftp.tile([128, 512], F32, tag="ft")
            nc.scalar.activation(ft[:], pf[:], AF.Identity, bias=b_sb[:])
            ft_r = ftp.tile([128, 512], F32R, tag="ft_r")
            nc.vector.tensor_copy(ft_r[:], ft[:])

            # replicated j-side factors: exp(-|X_j|^2/sq) etc. [128,512]
            reps = {}
            for name, src, nchunk, sigma in (
                ("x", xt, 8, sq),
                ("y", yt, 8, sq),
                ("f", ft_r, 1, sK),
            ):
                ps_n = tpsum.tile([128, 512], F32, tag="t")
                for kc in range(nchunk):
                    scr = scrp.tile([128, SHARD], F32R, tag="scr")
                    chunk = src[:, kc, :] if nchunk > 1 else src[:]
                    if kc % 3 == 2:
                        nc.scalar.activation(scr[:], chunk, AF.Square)
                    else:
                        nc.gpsimd.tensor_tensor(scr[:], chunk, chunk, OP.mult)
                    nc.tensor.matmul(
                        ps_n[:], onesmat_r[:], scr[:],
                        start=(kc == 0), stop=(kc == nchunk - 1),
                    )
                rep = repp.tile([128, 512], F32, tag=f"rep_{name}")
                nc.scalar.activation(rep[:], ps_n[:], AF.Exp, scale=-1.0 / sigma)
                reps[name] = rep

            colq_ps = cqp.tile([1, 512], F32, tag="cq")
            qr_tiles = []
            for ic in range(4):
                ics = slice(ic * 128, (ic + 1) * 128)
                gxx = gpsum.tile([128, 512], F32, tag="gxx")
                gyy = gpsum.tile([128, 512], F32, tag="gyy")
                gxy = gpsum.tile([128, 512], F32, tag="gxy")
                gf = gpsum.tile([128, 512], F32, tag="gf")
                for kc in range(8):
                    nc.tensor.matmul(
                        gxx[:], xst[:, kc, ics], xt[:, kc, :],
                        start=(kc == 0), stop=(kc == 7),
                    )
                for kc in range(8):
                    nc.tensor.matmul(
                        gyy[:], yst[:, kc, ics], yt[:, kc, :],
                        start=(kc == 0), stop=(kc == 7),
                    )
                for kc in range(8):
                    nc.tensor.matmul(
                        gxy[:], xst[:, kc, ics], yt[:, kc, :],
                        start=(kc == 0), stop=(kc == 7),
                    )
                nc.tensor.matmul(
                    gf[:], fsT_r[:, ics], ft_r[:], start=True, stop=True
                )

                ex = expp.tile([128, 512], F32, tag="ex")
                nc.scalar.activation(
                    ex[:], gxx[:], AF.Exp, bias=biases["x"][:, ic : ic + 1],
                    scale=2.0 / sq,
                )
                ey = expp.tile([128, 512], F32, tag="ey")
                nc.scalar.activation(
                    ey[:], gyy[:], AF.Exp, bias=biases["y"][:, ic : ic + 1],
                    scale=2.0 / sq,
                )
                exy = expp.tile([128, 512], F32, tag="exy")
                nc.scalar.activation(
                    exy[:], gxy[:], AF.Exp, bias=biases["x"][:, ic : ic + 1],
                    scale=2.0 / sq,
                )
                af = expp.tile([128, 512], F32, tag="af")
                nc.scalar.activation(
                    af[:], gf[:], AF.Exp, bias=biases["f"][:, ic : ic + 1],
                    scale=2.0 / sK,
                )

                col = b * 4 + ic
                # B = (1-eps)*af*cfrep + eps
                b0 = workp.tile([128, 512], F32, tag="b0")
                nc.vector.tensor_tensor(b0[:], af[:], reps["f"][:], OP.mult)
                nc.vector.tensor_scalar(
                    b0[:], b0[:], 1.0 - eps, eps, op0=OP.mult, op1=OP.add
                )
                # C = ex*cxrep + (ey - exy)*cyrep, built in place in ey/ex
                nc.vector.tensor_tensor(ey[:], ey[:], exy[:], OP.subtract)
                nc.vector.tensor_tensor(ey[:], ey[:], reps["y"][:], OP.mult)
                nc.vector.tensor_tensor(ex[:], ex[:], reps["x"][:], OP.mult)
                nc.vector.tensor_tensor(ey[:], ey[:], ex[:], OP.add)
                # P = B*C with fused row sum
                p_r = workp.tile([128, 512], F32R, tag="p_r")
                nc.vector.scalar_tensor_tensor(
                    p_r[:], b0[:], 1.0, ey[:], op0=OP.mult, op1=OP.mult,
                    accum_out=rowp_sb[:, col : col + 1],
                )
                # Q = B*exy*cyrep with fused row sum
                nc.vector.tensor_tensor(exy[:], exy[:], reps["y"][:], OP.mult)
                q_r = qrp.tile([128, 512], F32R, tag="q_r")
                nc.vector.scalar_tensor_tensor(
                    q_r[:], b0[:], 1.0, exy[:], op0=OP.mult, op1=OP.mult,
                    accum_out=rowq_sb[:, col : col + 1],
                )
                qr_tiles.append(q_r)
            # colQ partials: emitted after all four chains so the PE FIFO
            # never stalls mid-block waiting on the DVE chain
            for ic in range(4):
                nc.tensor.matmul(
                    colq_ps[:], ones_r[:], qr_tiles[ic][:],
                    start=(ic == 0), stop=(ic == 3), skip_group_check=True,
                )
            nc.vector.tensor_copy(colq_sb[0:1, j0 : j0 + 512], colq_ps[:])

        nc.sync.dma_start(rowp_d[:], rowp_sb[:])
        nc.sync.dma_start(rowq_d[:], rowq_sb[:])
        nc.sync.dma_start(colq_d[:], colq_sb[:])

    nc.compile()
    return nc


def kernel(X, Y, W, b, epsilon, sigma_K, sigma_q):
    global LAST_RESULTS
    X = np.ascontiguousarray(np.asarray(X, dtype=np.float32))
    Y = np.ascontiguousarray(np.asarray(Y, dtype=np.float32))
    W = np.ascontiguousarray(np.asarray(W, dtype=np.float32))
    b = np.ascontiguousarray(np.asarray(b, dtype=np.float32))
    eps = float(np.asarray(epsilon))
    sK = float(np.asarray(sigma_K))
    sq = float(np.asarray(sigma_q))

    key = (sq, sK, eps)
    if key not in _CACHE:
        _CACHE[key] = _build_program(sq, sK, eps)
    nc = _CACHE[key]

    XT = np.ascontiguousarray(X.T)
    YT = np.ascontiguousarray(Y.T)
    bcol = b.reshape(DF, 1)
    in_maps = []
    for c in range(NCORES):
        sl = slice(c * SHARD, (c + 1) * SHARD)
        in_maps.append(
            {
                "xst": np.ascontiguousarray(XT[:, sl]),
                "yst": np.ascontiguousarray(YT[:, sl]),
                "xft": XT,
                "yft": YT,
                "w": W,
                "bvec": bcol,
            }
        )

    try:
        res = bass_utils.run_bass_kernel_spmd(
            nc,
            in_maps,
            core_ids=list(range(NCORES)),
            trace=bool(os.environ.get("KERNEL_TRACE")),
        )
    except ModuleNotFoundError:
        # NTFF profiling hook unavailable in this container; run untraced.
        os.environ["BASS_NEVER_TRACE"] = "1"
        res = bass_utils.run_bass_kernel_spmd(
            nc, in_maps, core_ids=list(range(NCORES)), trace=False
        )
    LAST_RESULTS = res

    # rowp_sb[p, b*4+ic] holds block-b partial row sum for shard row ic*128+p
    rowp = np.zeros(N, dtype=np.float64)
    rowq = np.zeros(N, dtype=np.float64)
    colq = np.zeros(N, dtype=np.float64)
    for c in range(NCORES):
        rp = res.results[c]["rowp"].astype(np.float64)
        rq = res.results[c]["rowq"].astype(np.float64)
        rp = rp.reshape(128, NBLK, 4).sum(axis=1)  # [p, ic]
        rq = rq.reshape(128, NBLK, 4).sum(axis=1)
        base = c * SHARD
        for ic in range(4):
            rows = slice(base + ic * 128, base + (ic + 1) * 128)
            rowp[rows] = rp[:, ic]
            rowq[rows] = rq[:, ic]
        colq += res.results[c]["colq"].reshape(-1).astype(np.float64)

    # host combine in float64
    n = float(N)
    SP = rowp.sum()
    SQ = rowq.sum()
    X64 = X.astype(np.float64)
    Y64 = Y.astype(np.float64)
    dxy_diag = ((X64 - Y64) ** 2).sum(axis=1)
    tr_kxy = np.exp(-dxy_diag / sq).sum()
    # sum(Kx)+sum(Ky)-2*sum(Kxy) = SP - SQ ; traces of Kx,Ky are exactly n
    mmd2 = (SP - SQ - 2.0 * n + 2.0 * tr_kxy) / (n * (n - 1.0))
    row = (rowp - colq) / n
    v1 = float(row @ row) / n
    v2 = (SP - SQ) / (n * n)
    var_est = 4.0 * (v1 - v2 * v2)
    mmd_hat = -mmd2
    std = np.sqrt(var_est + 1e-8)
    return (
        np.float32(mmd_hat),
        np.float32(std),
        np.float32(mmd_hat / std),
    )


# revision 32
# speedup vs baseline: 1.0103x; 1.0103x over previous
"""Distributed Bass/Trainium2 kernel for the DKL MMD loss function.

Problem: nn_DKL_70970039599698. Full inputs X,Y [4096,1024], W [1024,128],
b [128], scalars epsilon/sigma_K/sigma_q. The reference computes (note it
uses featY for BOTH feature sides, so all three feature-distance matrices
are the same matrix Df):

  Kx  = (1-e)*exp(-Df/sK - Dxx/sq) + e*exp(-Dxx/sq)
  Ky  = (1-e)*exp(-Df/sK - Dyy/sq) + e*exp(-Dyy/sq)
  Kxy = (1-e)*exp(-Df/sK - Dxy/sq) + e*exp(-Dxy/sq)

and returns (-mmd2, sqrt(varEst+1e-8), ratio) built from sums/traces/row
sums of those three 4096x4096 matrices.

Sharding: each of the 8 cores owns a 512-row band i of the pair grid and
streams all 4096 columns j (512-column blocks). Everything needed
downstream is only:
  rowP_i = sum_j (Kx+Ky-Kxy)[i,j]     (per-core [128,32] partials)
  rowQ_i = sum_j Kxy[i,j]             (per-core [128,32] partials)
  colQ_j = sum_{i in shard} Kxy[i,j]  (per-core partial [1,4096])
which the host all-reduces / concatenates and combines in float64.

The host hands every tensor pre-transposed ([dim, rows] layout) so the
device never transposes: Gram operands DMA straight into SBUF in matmul
layout. Tiles are [i(128 part) x j(512 free)]: the i-side norm factor
rides the ACT exp bias slot, j-side factors are replicated [128,512]
tiles built by ones-matrix matmuls over gpsimd-squared chunks, row sums
come fused out of the DVE chain (scalar_tensor_tensor accum_out) and
col sums are per-block ones-vector matmuls. Grams run in float32r
(TF32, full PE rate; the PE rounds its inputs internally); norms stay
fp32. relu clamping of pdist2 is skipped on device: min off-diagonal
distances are >=1e-3 so the clamp only ever fires on the (noise-level)
diagonals, which the host handles exactly via trace terms.
"""

import os
from contextlib import ExitStack

import numpy as np

import concourse.bacc as bacc
import concourse.bass as bass
import concourse.mybir as mybir
import concourse.tile as tile
from concourse import bass_utils

N = 4096
D = 1024
DF = 128
NCORES = 8
SHARD = N // NCORES  # 512
NBLK = N // 512  # 8 column blocks of 512
F32 = mybir.dt.float32
F32R = mybir.dt.float32 if os.environ.get("NO_F32R") else mybir.dt.float32r
AF = mybir.ActivationFunctionType
OP = mybir.AluOpType

_CACHE: dict = {}
LAST_RESULTS = None  # BassKernelResults of the most recent run (for test.py)


def _build_program(sq: float, sK: float, eps: float):
    nc = bacc.Bacc(trn_type="TRN2", target_bir_lowering=False, debug=False)

    # all row-data inputs arrive pre-transposed: [dim, rows]
    xst_d = nc.dram_tensor("xst", [D, SHARD], F32R, kind="ExternalInput").ap()
    yst_d = nc.dram_tensor("yst", [D, SHARD], F32R, kind="ExternalInput").ap()
    xft_d = nc.dram_tensor("xft", [D, N], F32R, kind="ExternalInput").ap()
    yft_d = nc.dram_tensor("yft", [D, N], F32R, kind="ExternalInput").ap()
    w_d = nc.dram_tensor("w", [D, DF], F32R, kind="ExternalInput").ap()
    b_d = nc.dram_tensor("bvec", [DF, 1], F32, kind="ExternalInput").ap()
    rowp_d = nc.dram_tensor("rowp", [128, 4 * NBLK], F32, kind="ExternalOutput").ap()
    rowq_d = nc.dram_tensor("rowq", [128, 4 * NBLK], F32, kind="ExternalOutput").ap()
    colq_d = nc.dram_tensor("colq", [1, N], F32, kind="ExternalOutput").ap()

    use_pool = not os.environ.get("NO_POOL_CHAIN")

    with tile.TileContext(nc) as tc, ExitStack() as ctx:
        consts = ctx.enter_context(tc.tile_pool(name="consts", bufs=1))
        persist = ctx.enter_context(tc.tile_pool(name="persist", bufs=1))
        xtp = ctx.enter_context(tc.tile_pool(name="xtp", bufs=2))
        scrp = ctx.enter_context(tc.tile_pool(name="scrp", bufs=13))
        repp = ctx.enter_context(tc.tile_pool(name="repp", bufs=2))
        ftp = ctx.enter_context(tc.tile_pool(name="ftp", bufs=2))
        expp = ctx.enter_context(tc.tile_pool(name="expp", bufs=2))
        workp = ctx.enter_context(tc.tile_pool(name="workp", bufs=2))
        qrp = ctx.enter_context(tc.tile_pool(name="qrp", bufs=4))
        smallp = ctx.enter_context(tc.tile_pool(name="smallp", bufs=2))
        tpsum = ctx.enter_context(tc.tile_pool(name="tpsum", bufs=3, space="PSUM"))
        cqp = ctx.enter_context(tc.tile_pool(name="cqp", bufs=1, space="PSUM"))
        gpsum = ctx.enter_context(tc.tile_pool(name="gpsum", bufs=1, space="PSUM"))

        ones1 = consts.tile([1, 128], F32, tag="ones1")
        nc.gpsimd.memset(ones1[:], 1.0)
        ones128 = consts.tile([128, 1], F32, tag="ones128")
        nc.gpsimd.memset(ones128[:], 1.0)
        ones_r = consts.tile([128, 1], F32R, tag="ones_r")
        nc.vector.tensor_copy(ones_r[:], ones128[:])
        onesmat = consts.tile([128, 128], F32, tag="onesmat")
        nc.gpsimd.memset(onesmat[:], 1.0)
        onesmat_r = consts.tile([128, 128], F32R, tag="onesmat_r")
        nc.vector.tensor_copy(onesmat_r[:], onesmat[:])
        w_r = consts.tile([128, 8, 128], F32R, tag="w_r")
        nc.sync.dma_start(w_r[:], w_d.rearrange("(c p) f -> p c f", p=128))
        b_sb = consts.tile([128, 1], F32, tag="b_sb")
        nc.sync.dma_start(b_sb[:], b_d[:])

        # ---------------- phase 1: shard prep ----------------
        xst = persist.tile([128, 8, SHARD], F32R, tag="xst")  # [dim, kchunk, i]
        yst = persist.tile([128, 8, SHARD], F32R, tag="yst")
        for h in range(4):
            hs = slice(h * 2, (h + 1) * 2)
            nc.sync.dma_start(
                yst[:, hs, :],
                yst_d.rearrange("(c p) i -> p c i", p=128)[:, hs, :],
            )
        for h in range(4):
            hs = slice(h * 2, (h + 1) * 2)
            nc.sync.dma_start(
                xst[:, hs, :],
                xst_d.rearrange("(c p) i -> p c i", p=128)[:, hs, :],
            )

        # fsT: [feat, i] feature block for the shard (features of Y rows)
        fsT = persist.tile([128, SHARD], F32, tag="fsT")
        pf = tpsum.tile([128, SHARD], F32, tag="t")
        for kc in range(8):
            nc.tensor.matmul(
                pf[:], w_r[:, kc, :], yst[:, kc, :], start=(kc == 0), stop=(kc == 7)
            )
        nc.scalar.activation(fsT[:], pf[:], AF.Identity, bias=b_sb[:])
        fsT_r = persist.tile([128, SHARD], F32R, tag="fsT_r")
        nc.vector.tensor_copy(fsT_r[:], fsT[:])

        # shard norms -> [1,512] free -> per-i-chunk [128,1] exp biases
        biases = {}
        for name, src, nchunk, sigma in (
            ("x", xst, 8, sq),
            ("y", yst, 8, sq),
            ("f", fsT_r, 1, sK),
        ):
            ps_n = tpsum.tile([128, SHARD], F32, tag="t")
            for kc in range(nchunk):
                scr = scrp.tile([128, SHARD], F32R, tag="scr")
                chunk = src[:, kc, :] if nchunk > 1 else src[:]
                nc.gpsimd.tensor_tensor(scr[:], chunk, chunk, OP.mult)
                nc.tensor.matmul(
                    ps_n[0:1, :], ones_r[:], scr[:],
                    start=(kc == 0), stop=(kc == nchunk - 1),
                )
            nv = persist.tile([1, SHARD], F32, tag=f"nv_{name}")
            nc.vector.tensor_scalar_mul(nv[:], ps_n[0:1, :], -1.0 / sigma)
            # [1,512] -> 4x [128,1] via K=1 outer-product matmuls
            bias = persist.tile([128, 4], F32, tag=f"bias_{name}")
            for ic in range(4):
                ps_b = tpsum.tile([128, SHARD], F32, tag="t")
                nc.tensor.matmul(
                    ps_b[:, 0:1],
                    nv[0:1, ic * 128 : (ic + 1) * 128],
                    ones1[0:1, 0:1],
                    start=True,
                    stop=True,
                )
                nc.vector.tensor_copy(bias[:, ic : ic + 1], ps_b[:, 0:1])
            biases[name] = bias

        rowp_sb = persist.tile([128, 4 * NBLK], F32, tag="rowp_sb")
        rowq_sb = persist.tile([128, 4 * NBLK], F32, tag="rowq_sb")
        colq_sb = persist.tile([1, N], F32, tag="colq_sb")

        # ---------------- phase 2: stream column blocks ----------------
        for b in range(NBLK):
            j0 = b * 512
            xt = xtp.tile([128, 8, 512], F32R, tag="xt")  # [dim, kchunk, j]
            yt = xtp.tile([128, 8, 512], F32R, tag="yt")
            for h in range(4):
                hs = slice(h * 2, (h + 1) * 2)
                nc.sync.dma_start(
                    yt[:, hs, :],
                    yft_d[:, j0 : j0 + 512].rearrange(
                        "(c p) j -> p c j", p=128
                    )[:, hs, :],
                )
            for h in range(4):
                hs = slice(h * 2, (h + 1) * 2)
                nc.sync.dma_start(
                    xt[:, hs, :],
                    xft_d[:, j0 : j0 + 512].rearrange(
                        "(c p) j -> p c j", p=128
                    )[:, hs, :],
                )

            # F^T for this block ([feat, j])
            pf = tpsum.tile([128, 512], F32, tag="t")
            for kc in range(8):
                nc.tensor.matmul(
                    pf[:], w_r[:, kc, :], yt[:, kc, :], start=(kc == 0), stop=(kc == 7)
                )
            ft = ftp.tile([128, 512], F32, tag="ft")
            nc.scalar.activation(ft[:], pf[:], AF.Identity, bias=b_sb[:])
            ft_r = ftp.tile([128, 512], F32R, tag="ft_r")
            nc.vector.tensor_copy(ft_r[:], ft[:])

            # replicated j-side factors: exp(-|X_j|^2/sq) etc. [128,512]
            reps = {}
            for name, src, nchunk, sigma in (
                ("x", xt, 8, sq),
                ("y", yt, 8, sq),
                ("f", ft_r, 1, sK),
            ):
                ps_n = tpsum.tile([128, 512], F32, tag="t")
                for kc in range(nchunk):
                    scr = scrp.tile([128, SHARD], F32R, tag="scr")
                    chunk = src[:, kc, :] if nchunk > 1 else src[:]
                    if kc % 3 == 2:
                        nc.scalar.activation(scr[:], chunk, AF.Square)
                    else:
                        nc.gpsimd.tensor_tensor(scr[:], chunk, chunk, OP.mult)
                    nc.tensor.matmul(
                        ps_n[:], onesmat_r[:], scr[:],
                        start=(kc == 0), stop=(kc == nchunk - 1),
                    )
                rep = repp.tile([128, 512], F32, tag=f"rep_{name}")
                nc.scalar.activation(rep[:], ps_n[:], AF.Exp, scale=-1.0 / sigma)
                reps[name] = rep

            colq_ps = cqp.tile([1, 512], F32, tag="cq")
            qr_tiles = []
            for ic in range(4):
                ics = slice(ic * 128, (ic + 1) * 128)
                gxx = gpsum.tile([128, 512], F32, tag="gxx")
                gyy = gpsum.tile([128, 512], F32, tag="gyy")
                gxy = gpsum.tile([128, 512], F32, tag="gxy")
                gf = gpsum.tile([128, 512], F32, tag="gf")
                nc.tensor.matmul(
                    gf[:], fsT_r[:, ics], ft_r[:], start=True, stop=True
                )
                for kc in range(8):
                    nc.tensor.matmul(
                        gyy[:], yst[:, kc, ics], yt[:, kc, :],
                        start=(kc == 0), stop=(kc == 7),
                    )
                for kc in range(8):
                    nc.tensor.matmul(
                        gxy[:], xst[:, kc, ics], yt[:, kc, :],
                        start=(kc == 0), stop=(kc == 7),
                    )
                for kc in range(8):
                    nc.tensor.matmul(
                        gxx[:], xst[:, kc, ics], xt[:, kc, :],
                        start=(kc == 0), stop=(kc == 7),
                    )

                af = expp.tile([128, 512], F32, tag="af")
                nc.scalar.activation(
                    af[:], gf[:], AF.Exp, bias=biases["f"][:, ic : ic + 1],
                    scale=2.0 / sK,
                )
                ey = expp.tile([128, 512], F32, tag="ey")
                nc.scalar.activation(
                    ey[:], gyy[:], AF.Exp, bias=biases["y"][:, ic : ic + 1],
                    scale=2.0 / sq,
                )
                exy = expp.tile([128, 512], F32, tag="exy")
                nc.scalar.activation(
                    exy[:], gxy[:], AF.Exp, bias=biases["x"][:, ic : ic + 1],
                    scale=2.0 / sq,
                )
                ex = expp.tile([128, 512], F32, tag="ex")
                nc.scalar.activation(
                    ex[:], gxx[:], AF.Exp, bias=biases["x"][:, ic : ic + 1],
                    scale=2.0 / sq,
                )

                col = b * 4 + ic
                # B = (1-eps)*af*cfrep + eps
                b0 = workp.tile([128, 512], F32, tag="b0")
                nc.vector.tensor_tensor(b0[:], af[:], reps["f"][:], OP.mult)
                nc.vector.tensor_scalar(
                    b0[:], b0[:], 1.0 - eps, eps, op0=OP.mult, op1=OP.add
                )
                # C = ex*cxrep + (ey - exy)*cyrep, built in place in ey/ex
                nc.vector.tensor_tensor(ey[:], ey[:], exy[:], OP.subtract)
                nc.vector.tensor_tensor(ey[:], ey[:], reps["y"][:], OP.mult)
                nc.vector.tensor_tensor(ex[:], ex[:], reps["x"][:], OP.mult)
                nc.vector.tensor_tensor(ey[:], ey[:], ex[:], OP.add)
                # P = B*C with fused row sum
                p_r = workp.tile([128, 512], F32R, tag="p_r")
                nc.vector.scalar_tensor_tensor(
                    p_r[:], b0[:], 1.0, ey[:], op0=OP.mult, op1=OP.mult,
                    accum_out=rowp_sb[:, col : col + 1],
                )
                # Q = B*exy*cyrep with fused row sum
                nc.vector.tensor_tensor(exy[:], exy[:], reps["y"][:], OP.mult)
                q_r = qrp.tile([128, 512], F32R, tag="q_r")
                nc.vector.scalar_tensor_tensor(
                    q_r[:], b0[:], 1.0, exy[:], op0=OP.mult, op1=OP.mult,
                    accum_out=rowq_sb[:, col : col + 1],
                )
                qr_tiles.append(q_r)
            # colQ partials: emitted after all four chains so the PE FIFO
            # never stalls mid-block waiting on the DVE chain
            for ic in range(4):
                nc.tensor.matmul(
                    colq_ps[:], ones_r[:], qr_tiles[ic][:],
                    start=(ic == 0), stop=(ic == 3), skip_group_check=True,
                )
            nc.vector.tensor_copy(colq_sb[0:1, j0 : j0 + 512], colq_ps[:])

        nc.sync.dma_start(rowp_d[:], rowp_sb[:])
        nc.sync.dma_start(rowq_d[:], rowq_sb[:])
        nc.sync.dma_start(colq_d[:], colq_sb[:])

    nc.compile()
    return nc


def kernel(X, Y, W, b, epsilon, sigma_K, sigma_q):
    global LAST_RESULTS
    X = np.ascontiguousarray(np.asarray(X, dtype=np.float32))
    Y = np.ascontiguousarray(np.asarray(Y, dtype=np.float32))
    W = np.ascontiguousarray(np.asarray(W, dtype=np.float32))
    b = np.ascontiguousarray(np.asarray(b, dtype=np.float32))
    eps = float(np.asarray(epsilon))
    sK = float(np.asarray(sigma_K))
    sq = float(np.asarray(sigma_q))

    key = (sq, sK, eps)
    if key not in _CACHE:
        _CACHE[key] = _build_program(sq, sK, eps)
    nc = _CACHE[key]

    XT = np.ascontiguousarray(X.T)
    YT = np.ascontiguousarray(Y.T)
    bcol = b.reshape(DF, 1)
    in_maps = []
    for c in range(NCORES):
        sl = slice(c * SHARD, (c + 1) * SHARD)
        in_maps.append(
            {
                "xst": np.ascontiguousarray(XT[:, sl]),
                "yst": np.ascontiguousarray(YT[:, sl]),
                "xft": XT,
                "yft": YT,
                "w": W,
                "bvec": bcol,
            }
        )

    try:
        res = bass_utils.run_bass_kernel_spmd(
            nc,
            in_maps,
            core_ids=list(range(NCORES)),
            trace=bool(os.environ.get("KERNEL_TRACE")),
        )
    except ModuleNotFoundError:
        # NTFF profiling hook unavailable in this container; run untraced.
        os.environ["BASS_NEVER_TRACE"] = "1"
        res = bass_utils.run_bass_kernel_spmd(
            nc, in_maps, core_ids=list(range(NCORES)), trace=False
        )
    LAST_RESULTS = res

    # rowp_sb[p, b*4+ic] holds block-b partial row sum for shard row ic*128+p
    rowp = np.zeros(N, dtype=np.float64)
    rowq = np.zeros(N, dtype=np.float64)
    colq = np.zeros(N, dtype=np.float64)
    for c in range(NCORES):
        rp = res.results[c]["rowp"].astype(np.float64)
        rq = res.results[c]["rowq"].astype(np.float64)
        rp = rp.reshape(128, NBLK, 4).sum(axis=1)  # [p, ic]
        rq = rq.reshape(128, NBLK, 4).sum(axis=1)
        base = c * SHARD
        for ic in range(4):
            rows = slice(base + ic * 128, base + (ic + 1) * 128)
            rowp[rows] = rp[:, ic]
            rowq[rows] = rq[:, ic]
        colq += res.results[c]["colq"].reshape(-1).astype(np.float64)

    # host combine in float64
    n = float(N)
    SP = rowp.sum()
    SQ = rowq.sum()
    X64 = X.astype(np.float64)
    Y64 = Y.astype(np.float64)
    dxy_diag = ((X64 - Y64) ** 2).sum(axis=1)
    tr_kxy = np.exp(-dxy_diag / sq).sum()
    # sum(Kx)+sum(Ky)-2*sum(Kxy) = SP - SQ ; traces of Kx,Ky are exactly n
    mmd2 = (SP - SQ - 2.0 * n + 2.0 * tr_kxy) / (n * (n - 1.0))
    row = (rowp - colq) / n
    v1 = float(row @ row) / n
    v2 = (SP - SQ) / (n * n)
    var_est = 4.0 * (v1 - v2 * v2)
    mmd_hat = -mmd2
    std = np.sqrt(var_est + 1e-8)
    return (
        np.float32(mmd_hat),
        np.float32(std),
        np.float32(mmd_hat / std),
    )
